# revision 31
# baseline (speedup 1.0000x reference)
"""AdaptiveSinLayer kernel for 8 TRN2 NeuronCores (data-parallel).

out[t] = sin(OMEGA*(x[t] @ weight[indices[t]] + bias)).

v9: compact deduped weight stream + dynamic-rhs matmul. The kernel is
HBM-byte bound (~322 GB/s/core; x 16MB + w 16MB + out 16MB = 48MB ->
~156us pure DMA), so the win is cutting weight bytes. Tiles are
globally sorted by channel on the host and dealt to cores in contiguous
128-tile blocks, so each core needs only ~81-91 distinct weights. Those
uniques are host-packed into a compact stream (first-use order), padded
to U_PAD slots, and DMAed by N_CHUNK chunk loads (4KB descriptors) that
are predicated (cond= -> OOB-skip) on a per-core chunk count, so only
~11.5MB of weights move. Each tile's matmul reads its slot via a
DYNAMIC rhs column offset (slot id TensorLoad'ed into a PE register) --
weights must be the MOVING operand since ldweights needs static
offsets, so x is the stationary side: psum[p_half, O] = x_chunk.T @
w_slot, accumulated over k plus a K=1 ones-row matmul that adds the
bias row. Total ~43.5MB -> ~137us floor.

Math (x, w, b pre-scaled by OMEGA/2pi so the sine period in psum units
is exactly 1): psum = v = (omega/2pi)(x@W + b), then
  u = v + C        (magic round: u - C = round(v))
  d = (u - C) - v  (fused scalar_tensor_tensor, in [-0.5, 0.5])
  o = Sin(-2pi*d)  (single op per group; arg within [-pi, pi])
Pointwise ops run over G=4 tiles at once; Sin + out DMA for group g
issue after u/d of group g+1 so ACT's FIFO never stalls; u runs on ACT
except every 3rd group (DVE).
"""
import numpy as np
import ml_dtypes
from contextlib import ExitStack

from concourse import bacc, bass, mybir, tile
from concourse.bass_utils import run_bass_kernel_spmd

N_CORES = 8
T, P, I, O, N_CH = 1024, 256, 256, 256, 1024
T_SH = T // N_CORES
OMEGA = 30.0
TWO_PI = float(2 * np.pi)
C_MAGIC = float(1.5 * 2**23)

BF16 = mybir.dt.bfloat16
F32 = mybir.dt.float32
I32 = mybir.dt.int32
FCOLS = T_SH * 512

U_PAD = 96           # compact weight stream capacity (slots of 512 cols)
CHUNK = 4            # slots per predicated chunk load
N_CHUNK = U_PAD // CHUNK


def build_nc(repeat=1, g=4, x_bufs=4, o_bufs=3,
             u_bufs=4, d_bufs=2, psum_bufs=2,
             out_engine="scalar", ld_engine="sync",
             out_bf16=True, u_dve_every=3, ld_span=1, dma_only=False,
             unroll=1, no_bias=False, static_slot=False,
             chunk_engine="sync", v_engine="vector", u_engine="gpsimd",
             d_bf16=True):
    G = g
    N_G = T_SH // G
    nc = bacc.Bacc(None, target_bir_lowering=False)
    xT = nc.declare_dram_parameter("xT", [128, FCOLS], BF16, isOutput=False)
    wu = nc.declare_dram_parameter("wu", [128, U_PAD * 512], BF16,
                                   isOutput=False)
    bt = nc.declare_dram_parameter("bt", [128, g * 512], F32, isOutput=False)
    st = nc.declare_dram_parameter("st", [1, T_SH], I32, isOutput=False)
    cf = nc.declare_dram_parameter("cf", [1, N_CHUNK], I32, isOutput=False)
    out_dt = BF16 if out_bf16 else F32
    if dma_only:
        out = nc.declare_dram_parameter("out", [128, FCOLS], BF16,
                                        isOutput=True)
    else:
        out = nc.declare_dram_parameter(
            "out", [128, T_SH, 2, 256], out_dt, isOutput=True)

    with tile.TileContext(nc) as tc, ExitStack() as ctx:
        const_pool = ctx.enter_context(tc.tile_pool(name="const", bufs=1))
        x_pool = ctx.enter_context(tc.tile_pool(name="x", bufs=x_bufs))
        u_pool = ctx.enter_context(tc.tile_pool(name="u", bufs=u_bufs))
        d_pool = ctx.enter_context(tc.tile_pool(name="d", bufs=d_bufs))
        o_pool = ctx.enter_context(tc.tile_pool(name="o", bufs=o_bufs))
        psum_pool = ctx.enter_context(
            tc.tile_pool(name="psum", bufs=psum_bufs, space="PSUM"))

        bt_sb = const_pool.tile([128, g * 512], F32)
        nc.sync.dma_start(bt_sb[:], bt[:])
        st_sb = const_pool.tile([1, T_SH], I32)
        nc.sync.dma_start(st_sb[:], st[:])
        cf_sb = const_pool.tile([1, N_CHUNK], I32)
        nc.sync.dma_start(cf_sb[:], cf[:])
        c_magic = const_pool.tile([128, 1], F32)
        nc.gpsimd.memset(c_magic[:], C_MAGIC)
        wu_sb = const_pool.tile([128, U_PAD * 512], BF16)

        reg_n = [0]
        CH_ENG = {"sync": mybir.EngineType.SP,
                  "scalar": mybir.EngineType.Activation,
                  "gpsimd": mybir.EngineType.Pool}

        def chunk_load(c, eng="scalar"):
            reg_n[0] += 1
            reg = nc.alloc_registers(f"cf{reg_n[0]}", [CH_ENG[eng]])
            nc.regs_load(reg, cf_sb[0:1, c : c + 1])
            cond = nc.snap(reg, donate=True, min_val=0, max_val=1)
            cols = slice(c * CHUNK * 512, (c + 1) * CHUNK * 512)
            getattr(nc, eng).dma_start(wu_sb[:, cols], wu[:, cols], cond=cond)

        pending = []  # [(ob, dB, t0)] sin+store lagged one group

        def flush_pending():
            ob, dB, t0 = pending.pop()
            nc.scalar.activation(
                ob[:], dB[:], mybir.ActivationFunctionType.Sin,
                scale=-TWO_PI)
            getattr(nc, out_engine).dma_start(
                out[:, t0 : t0 + G], ob[:])

        ld_state = {}

        def group_body(gi):
            t0 = gi * G
            cols = slice(t0 * 512, (t0 + G) * 512)
            if gi + 1 < N_CHUNK:
                chunk_load(gi + 1, eng=chunk_engine)
            if gi % ld_span == 0:
                spc = slice(t0 * 512, (t0 + ld_span * G) * 512)
                xbig = x_pool.tile([128, ld_span * G * 512], BF16)
                getattr(nc, ld_engine).dma_start(xbig[:], xT[:, spc])
                ld_state["x"] = xbig
            off = (gi % ld_span) * G * 512
            xb = ld_state["x"]
            if dma_only:
                getattr(nc, out_engine).dma_start(
                    out[:, cols], xb[:, off : off + G * 512])
                return

            psum = psum_pool.tile([128, G, 2, 256], F32)
            for j in range(G):
                p = t0 + j
                if static_slot:
                    sv = None
                else:
                    reg_n[0] += 1
                    reg = nc.alloc_registers(f"st{reg_n[0]}",
                                             [mybir.EngineType.PE])
                    nc.regs_load(reg, st_sb[0:1, p : p + 1])
                    sv = nc.snap(reg, donate=True, min_val=0,
                                 max_val=U_PAD - 1)
                trk = min(p, U_PAD - 1) * 512
                for h in range(2):
                    c0 = off + j * 512
                    for k in range(2):
                        if static_slot:
                            cw = (p % U_PAD) * 512 + k * 256
                            rhs = wu_sb[:, cw : cw + 256]
                        else:
                            a = wu_sb[:, bass.ds(sv * 512 + k * 256, 256)]
                            # slots are monotone (st[p] <= p), so tracking
                            # the read at slot min(p, U_PAD-1) is safe and
                            # narrows the dep to one chunk load.
                            rhs = bass.AP(
                                tensor=a.tensor, offset=a.offset, ap=a.ap,
                                dep_tracking_offset=trk + k * 256)
                        nc.tensor.matmul(
                            psum[:, j, h, :],
                            xb[:, c0 + 256 * k + 128 * h :
                                  c0 + 256 * k + 128 * (h + 1)],
                            rhs,
                            start=(k == 0),
                            stop=(k == 1),
                        )
            if no_bias:
                vB = psum
            else:
                vB = u_pool.tile([128, G, 2, 256], F32)
                getattr(nc, v_engine).tensor_tensor(
                    vB[:], psum[:], bt_sb[:], mybir.AluOpType.add)
            uB = u_pool.tile([128, G, 2, 256], F32)
            ue = (u_engine if not isinstance(u_engine, (list, tuple))
                  else u_engine[gi % len(u_engine)])
            if ue == "scalar":
                nc.scalar.activation(
                    uB[:], vB[:], mybir.ActivationFunctionType.Identity,
                    bias=c_magic[:], scale=1.0)
            else:
                getattr(nc, ue).tensor_scalar(
                    uB[:], vB[:], C_MAGIC, None, mybir.AluOpType.add)
            dB = d_pool.tile([128, G, 2, 256], BF16 if d_bf16 else F32)
            nc.vector.scalar_tensor_tensor(
                dB[:], uB[:], C_MAGIC, vB[:],
                mybir.AluOpType.subtract, mybir.AluOpType.subtract)
            ob = o_pool.tile([128, G, 2, 256], out_dt)
            pending.append((ob, dB, t0))

        def full_body(_iv=None):
            chunk_load(0, eng=chunk_engine)
            for gi in range(N_G):
                group_body(gi)
                if len(pending) > 1:
                    flush_pending()
            while pending:
                flush_pending()

        if repeat == 1:
            for _ in range(unroll):
                full_body()
        else:
            assert repeat % unroll == 0
            with tc.For_i(0, repeat // unroll, 1):
                for _ in range(unroll):
                    full_body()

    nc.compile()
    return nc


_NC = None


def _get_nc():
    global _NC
    if _NC is None:
        _NC = build_nc()
    return _NC


def _schedule(indices):
    """Sort tiles by channel globally, deal contiguous 128-tile blocks to
    cores. Per core: sched (original tile ids in schedule order), slot id
    per position (first-use order), unique channel list."""
    order = np.argsort(indices, kind="stable")
    scheds, slots, uniqs = [], [], []
    for c in range(N_CORES):
        blk = order[c * T_SH : (c + 1) * T_SH]
        ch = indices[blk]
        slot = np.empty(T_SH, np.int32)
        uniq = []
        for p in range(T_SH):
            if p == 0 or ch[p] != ch[p - 1]:
                uniq.append(ch[p])
            slot[p] = len(uniq) - 1
        scheds.append(blk)
        slots.append(slot)
        uniqs.append(np.asarray(uniq, np.int64))
    return scheds, slots, uniqs


_SCHEDS = None  # stashed by make_in_maps for unshard


def make_in_maps(x, weight, bias, indices):
    global _SCHEDS
    x = np.asarray(x, dtype=np.float32)
    weight = np.asarray(weight, dtype=np.float32)
    bias = np.asarray(bias, dtype=np.float32).reshape(O)
    indices = np.asarray(indices).astype(np.int64)

    bprime = (OMEGA / TWO_PI * bias).astype(np.float32)  # [O]
    bt_h = np.ascontiguousarray(
        np.broadcast_to(np.tile(bprime, 4 * 2), (128, 4 * 512))
    ).astype(np.float32)
    wsc = (OMEGA / TWO_PI * weight).astype(np.float32)

    scheds, slots, uniqs = _schedule(indices)
    assert max(len(u) for u in uniqs) <= U_PAD, (
        "unique-channel overflow; pathological index distribution")
    _SCHEDS = scheds
    in_maps = []
    for c in range(N_CORES):
        sched = scheds[c]
        xT_h = (
            np.ascontiguousarray(
                x[sched].reshape(T_SH, P, 2, 128).transpose(3, 0, 2, 1))
            .astype(ml_dtypes.bfloat16)
            .reshape(128, FCOLS)
        )
        uq = uniqs[c]
        wu_h = np.zeros((128, U_PAD * 512), ml_dtypes.bfloat16)
        ws = wsc[uq]  # [U_c, I, O]
        wu_h[:, : len(uq) * 512] = (
            np.ascontiguousarray(
                ws.reshape(len(uq), 2, 128, O).transpose(2, 0, 1, 3))
            .astype(ml_dtypes.bfloat16)
            .reshape(128, len(uq) * 512)
        )
        n_chunk = -(-len(uq) // CHUNK)
        cf_h = np.zeros((1, N_CHUNK), np.int32)
        cf_h[0, :n_chunk] = 1
        in_maps.append({"xT": xT_h, "wu": wu_h, "bt": bt_h,
                        "st": slots[c].reshape(1, T_SH), "cf": cf_h})
    return in_maps


def unshard(results):
    out = np.empty((T, P, O), np.float32)
    for c, r in enumerate(results):
        o = np.asarray(r["out"]).astype(np.float32)  # [128, T_SH, 2, 256]
        # dims: (point-in-half, tile, point-half, feature)
        o = o.transpose(1, 2, 0, 3).reshape(T_SH, P, O)
        out[_SCHEDS[c]] = o
    return out


def kernel(x, weight, bias, indices):
    nc = _get_nc()
    in_maps = make_in_maps(x, weight, bias, indices)
    try:
        res = run_bass_kernel_spmd(nc, in_maps, core_ids=list(range(N_CORES)))
    except ModuleNotFoundError:
        import os

        os.environ["BASS_NEVER_TRACE"] = "1"
        res = run_bass_kernel_spmd(nc, in_maps, core_ids=list(range(N_CORES)))
    return unshard(res.results)


# revision 32
# speedup vs baseline: 4.6197x; 4.6197x over previous
"""AdaptiveSinLayer kernel for 8 TRN2 NeuronCores (data-parallel).

out[t] = sin(OMEGA*(x[t] @ weight[indices[t]] + bias)).

v9: compact deduped weight stream + dynamic-rhs matmul. The kernel is
HBM-byte bound (~322 GB/s/core; x 16MB + w 16MB + out 16MB = 48MB ->
~156us pure DMA), so the win is cutting weight bytes. Tiles are
globally sorted by channel on the host and dealt to cores in contiguous
128-tile blocks, so each core needs only ~81-91 distinct weights. Those
uniques are host-packed into a compact stream (first-use order), padded
to U_PAD slots, and DMAed by N_CHUNK chunk loads (4KB descriptors) that
are predicated (cond= -> OOB-skip) on a per-core chunk count, so only
~11.5MB of weights move. Each tile's matmul reads its slot via a
DYNAMIC rhs column offset (slot id TensorLoad'ed into a PE register) --
weights must be the MOVING operand since ldweights needs static
offsets, so x is the stationary side: psum[p_half, O] = x_chunk.T @
w_slot, accumulated over k plus a K=1 ones-row matmul that adds the
bias row. Total ~43.5MB -> ~137us floor.

Math (x, w, b pre-scaled by OMEGA/2pi so the sine period in psum units
is exactly 1): psum = v = (omega/2pi)(x@W + b), then
  u = v + C        (magic round: u - C = round(v))
  d = (u - C) - v  (fused scalar_tensor_tensor, in [-0.5, 0.5])
  o = Sin(-2pi*d)  (single op per group; arg within [-pi, pi])
Pointwise ops run over G=4 tiles at once; Sin + out DMA for group g
issue after u/d of group g+1 so ACT's FIFO never stalls; u runs on ACT
except every 3rd group (DVE).
"""
import numpy as np
import ml_dtypes
from contextlib import ExitStack

from concourse import bacc, bass, mybir, tile
from concourse.bass_utils import run_bass_kernel_spmd

N_CORES = 8
T, P, I, O, N_CH = 1024, 256, 256, 256, 1024
T_SH = T // N_CORES
OMEGA = 30.0
TWO_PI = float(2 * np.pi)
C_MAGIC = float(1.5 * 2**23)

BF16 = mybir.dt.bfloat16
F32 = mybir.dt.float32
I32 = mybir.dt.int32
FCOLS = T_SH * 512

U_PAD = 96           # compact weight stream capacity (slots of 512 cols)
CHUNK = 4            # slots per predicated chunk load
N_CHUNK = U_PAD // CHUNK


def build_nc(repeat=1, g=4, x_bufs=4, o_bufs=3,
             u_bufs=4, d_bufs=2, psum_bufs=2,
             out_engine="scalar", ld_engine="sync",
             out_bf16=True, u_dve_every=3, ld_span=1, dma_only=False,
             unroll=1, no_bias=False, static_slot=False,
             chunk_engine="sync", v_engine="vector",
             u_engine=("scalar", "scalar", "vector"),
             d_bf16=True):
    G = g
    N_G = T_SH // G
    nc = bacc.Bacc(None, target_bir_lowering=False)
    xT = nc.declare_dram_parameter("xT", [128, FCOLS], BF16, isOutput=False)
    wu = nc.declare_dram_parameter("wu", [128, U_PAD * 512], BF16,
                                   isOutput=False)
    bt = nc.declare_dram_parameter("bt", [128, g * 512], F32, isOutput=False)
    st = nc.declare_dram_parameter("st", [1, T_SH], I32, isOutput=False)
    cf = nc.declare_dram_parameter("cf", [1, N_CHUNK], I32, isOutput=False)
    out_dt = BF16 if out_bf16 else F32
    if dma_only:
        out = nc.declare_dram_parameter("out", [128, FCOLS], BF16,
                                        isOutput=True)
    else:
        out = nc.declare_dram_parameter(
            "out", [128, T_SH, 2, 256], out_dt, isOutput=True)

    with tile.TileContext(nc) as tc, ExitStack() as ctx:
        const_pool = ctx.enter_context(tc.tile_pool(name="const", bufs=1))
        x_pool = ctx.enter_context(tc.tile_pool(name="x", bufs=x_bufs))
        u_pool = ctx.enter_context(tc.tile_pool(name="u", bufs=u_bufs))
        d_pool = ctx.enter_context(tc.tile_pool(name="d", bufs=d_bufs))
        o_pool = ctx.enter_context(tc.tile_pool(name="o", bufs=o_bufs))
        psum_pool = ctx.enter_context(
            tc.tile_pool(name="psum", bufs=psum_bufs, space="PSUM"))

        bt_sb = const_pool.tile([128, g * 512], F32)
        nc.sync.dma_start(bt_sb[:], bt[:])
        st_sb = const_pool.tile([1, T_SH], I32)
        nc.sync.dma_start(st_sb[:], st[:])
        cf_sb = const_pool.tile([1, N_CHUNK], I32)
        nc.sync.dma_start(cf_sb[:], cf[:])
        c_magic = const_pool.tile([128, 1], F32)
        nc.gpsimd.memset(c_magic[:], C_MAGIC)
        wu_sb = const_pool.tile([128, U_PAD * 512], BF16)

        reg_n = [0]
        CH_ENG = {"sync": mybir.EngineType.SP,
                  "scalar": mybir.EngineType.Activation,
                  "gpsimd": mybir.EngineType.Pool}

        def chunk_load(c, eng="scalar"):
            reg_n[0] += 1
            reg = nc.alloc_registers(f"cf{reg_n[0]}", [CH_ENG[eng]])
            nc.regs_load(reg, cf_sb[0:1, c : c + 1])
            cond = nc.snap(reg, donate=True, min_val=0, max_val=1)
            cols = slice(c * CHUNK * 512, (c + 1) * CHUNK * 512)
            getattr(nc, eng).dma_start(wu_sb[:, cols], wu[:, cols], cond=cond)

        pending = []  # [(ob, dB, t0)] sin+store lagged one group

        def flush_pending():
            ob, dB, t0 = pending.pop()
            nc.scalar.activation(
                ob[:], dB[:], mybir.ActivationFunctionType.Sin,
                scale=-TWO_PI)
            getattr(nc, out_engine).dma_start(
                out[:, t0 : t0 + G], ob[:])

        ld_state = {}

        def group_body(gi):
            t0 = gi * G
            cols = slice(t0 * 512, (t0 + G) * 512)
            if gi + 1 < N_CHUNK:
                chunk_load(gi + 1, eng=chunk_engine)
            if gi % ld_span == 0:
                spc = slice(t0 * 512, (t0 + ld_span * G) * 512)
                xbig = x_pool.tile([128, ld_span * G * 512], BF16)
                getattr(nc, ld_engine).dma_start(xbig[:], xT[:, spc])
                ld_state["x"] = xbig
            off = (gi % ld_span) * G * 512
            xb = ld_state["x"]
            if dma_only:
                getattr(nc, out_engine).dma_start(
                    out[:, cols], xb[:, off : off + G * 512])
                return

            psum = psum_pool.tile([128, G, 2, 256], F32)
            for j in range(G):
                p = t0 + j
                if static_slot:
                    sv = None
                else:
                    reg_n[0] += 1
                    reg = nc.alloc_registers(f"st{reg_n[0]}",
                                             [mybir.EngineType.PE])
                    nc.regs_load(reg, st_sb[0:1, p : p + 1])
                    sv = nc.snap(reg, donate=True, min_val=0,
                                 max_val=U_PAD - 1)
                trk = min(p, U_PAD - 1) * 512
                for h in range(2):
                    c0 = off + j * 512
                    for k in range(2):
                        if static_slot:
                            cw = (p % U_PAD) * 512 + k * 256
                            rhs = wu_sb[:, cw : cw + 256]
                        else:
                            a = wu_sb[:, bass.ds(sv * 512 + k * 256, 256)]
                            # slots are monotone (st[p] <= p), so tracking
                            # the read at slot min(p, U_PAD-1) is safe and
                            # narrows the dep to one chunk load.
                            rhs = bass.AP(
                                tensor=a.tensor, offset=a.offset, ap=a.ap,
                                dep_tracking_offset=trk + k * 256)
                        nc.tensor.matmul(
                            psum[:, j, h, :],
                            xb[:, c0 + 256 * k + 128 * h :
                                  c0 + 256 * k + 128 * (h + 1)],
                            rhs,
                            start=(k == 0),
                            stop=(k == 1),
                        )
            if no_bias:
                vB = psum
            else:
                vB = u_pool.tile([128, G, 2, 256], F32)
                getattr(nc, v_engine).tensor_tensor(
                    vB[:], psum[:], bt_sb[:], mybir.AluOpType.add)
            uB = u_pool.tile([128, G, 2, 256], F32)
            ue = (u_engine if not isinstance(u_engine, (list, tuple))
                  else u_engine[gi % len(u_engine)])
            if ue == "scalar":
                nc.scalar.activation(
                    uB[:], vB[:], mybir.ActivationFunctionType.Identity,
                    bias=c_magic[:], scale=1.0)
            else:
                getattr(nc, ue).tensor_scalar(
                    uB[:], vB[:], C_MAGIC, None, mybir.AluOpType.add)
            dB = d_pool.tile([128, G, 2, 256], BF16 if d_bf16 else F32)
            nc.vector.scalar_tensor_tensor(
                dB[:], uB[:], C_MAGIC, vB[:],
                mybir.AluOpType.subtract, mybir.AluOpType.subtract)
            ob = o_pool.tile([128, G, 2, 256], out_dt)
            pending.append((ob, dB, t0))

        def full_body(_iv=None):
            chunk_load(0, eng=chunk_engine)
            for gi in range(N_G):
                group_body(gi)
                if len(pending) > 1:
                    flush_pending()
            while pending:
                flush_pending()

        if repeat == 1:
            for _ in range(unroll):
                full_body()
        else:
            assert repeat % unroll == 0
            with tc.For_i(0, repeat // unroll, 1):
                for _ in range(unroll):
                    full_body()

    nc.compile()
    return nc


_NC = None


def _get_nc():
    global _NC
    if _NC is None:
        _NC = build_nc()
    return _NC


def _schedule(indices):
    """Sort tiles by channel globally, deal contiguous 128-tile blocks to
    cores. Per core: sched (original tile ids in schedule order), slot id
    per position (first-use order), unique channel list."""
    order = np.argsort(indices, kind="stable")
    scheds, slots, uniqs = [], [], []
    for c in range(N_CORES):
        blk = order[c * T_SH : (c + 1) * T_SH]
        ch = indices[blk]
        slot = np.empty(T_SH, np.int32)
        uniq = []
        for p in range(T_SH):
            if p == 0 or ch[p] != ch[p - 1]:
                uniq.append(ch[p])
            slot[p] = len(uniq) - 1
        scheds.append(blk)
        slots.append(slot)
        uniqs.append(np.asarray(uniq, np.int64))
    return scheds, slots, uniqs


_SCHEDS = None  # stashed by make_in_maps for unshard


def make_in_maps(x, weight, bias, indices):
    global _SCHEDS
    x = np.asarray(x, dtype=np.float32)
    weight = np.asarray(weight, dtype=np.float32)
    bias = np.asarray(bias, dtype=np.float32).reshape(O)
    indices = np.asarray(indices).astype(np.int64)

    bprime = (OMEGA / TWO_PI * bias).astype(np.float32)  # [O]
    bt_h = np.ascontiguousarray(
        np.broadcast_to(np.tile(bprime, 4 * 2), (128, 4 * 512))
    ).astype(np.float32)
    wsc = (OMEGA / TWO_PI * weight).astype(np.float32)

    scheds, slots, uniqs = _schedule(indices)
    assert max(len(u) for u in uniqs) <= U_PAD, (
        "unique-channel overflow; pathological index distribution")
    _SCHEDS = scheds
    in_maps = []
    for c in range(N_CORES):
        sched = scheds[c]
        xT_h = (
            np.ascontiguousarray(
                x[sched].reshape(T_SH, P, 2, 128).transpose(3, 0, 2, 1))
            .astype(ml_dtypes.bfloat16)
            .reshape(128, FCOLS)
        )
        uq = uniqs[c]
        wu_h = np.zeros((128, U_PAD * 512), ml_dtypes.bfloat16)
        ws = wsc[uq]  # [U_c, I, O]
        wu_h[:, : len(uq) * 512] = (
            np.ascontiguousarray(
                ws.reshape(len(uq), 2, 128, O).transpose(2, 0, 1, 3))
            .astype(ml_dtypes.bfloat16)
            .reshape(128, len(uq) * 512)
        )
        n_chunk = -(-len(uq) // CHUNK)
        cf_h = np.zeros((1, N_CHUNK), np.int32)
        cf_h[0, :n_chunk] = 1
        in_maps.append({"xT": xT_h, "wu": wu_h, "bt": bt_h,
                        "st": slots[c].reshape(1, T_SH), "cf": cf_h})
    return in_maps


def unshard(results):
    out = np.empty((T, P, O), np.float32)
    for c, r in enumerate(results):
        o = np.asarray(r["out"]).astype(np.float32)  # [128, T_SH, 2, 256]
        # dims: (point-in-half, tile, point-half, feature)
        o = o.transpose(1, 2, 0, 3).reshape(T_SH, P, O)
        out[_SCHEDS[c]] = o
    return out


def kernel(x, weight, bias, indices):
    nc = _get_nc()
    in_maps = make_in_maps(x, weight, bias, indices)
    try:
        res = run_bass_kernel_spmd(nc, in_maps, core_ids=list(range(N_CORES)))
    except ModuleNotFoundError:
        import os

        os.environ["BASS_NEVER_TRACE"] = "1"
        res = run_bass_kernel_spmd(nc, in_maps, core_ids=list(range(N_CORES)))
    return unshard(res.results)


# revision 44
# speedup vs baseline: 5.5012x; 1.1908x over previous
"""AdaptiveSinLayer kernel for 8 TRN2 NeuronCores (data-parallel).

out[t] = sin(OMEGA*(x[t] @ weight[indices[t]] + bias)).

v9: compact deduped weight stream + dynamic-rhs matmul. The kernel is
HBM-byte bound (~322 GB/s/core; x 16MB + w 16MB + out 16MB = 48MB ->
~156us pure DMA), so the win is cutting weight bytes. Tiles are
globally sorted by channel on the host and dealt to cores in contiguous
128-tile blocks, so each core needs only ~81-91 distinct weights. Those
uniques are host-packed into a compact stream (first-use order), padded
to U_PAD slots, and DMAed by N_CHUNK chunk loads (4KB descriptors) that
are predicated (cond= -> OOB-skip) on a per-core chunk count, so only
~11.5MB of weights move. Each tile's matmul reads its slot via a
DYNAMIC rhs column offset (slot id TensorLoad'ed into a PE register) --
weights must be the MOVING operand since ldweights needs static
offsets, so x is the stationary side: psum[p_half, O] = x_chunk.T @
w_slot, accumulated over k plus a K=1 ones-row matmul that adds the
bias row. Total ~43.5MB -> ~137us floor.

Math (x, w, b pre-scaled by OMEGA/2pi so the sine period in psum units
is exactly 1): psum = v = (omega/2pi)(x@W + b), then
  u = v + C        (magic round: u - C = round(v))
  d = (u - C) - v  (fused scalar_tensor_tensor, in [-0.5, 0.5])
  o = Sin(-2pi*d)  (single op per group; arg within [-pi, pi])
Pointwise ops run over G=4 tiles at once; Sin + out DMA for group g
issue after u/d of group g+1 so ACT's FIFO never stalls; u runs on ACT
except every 3rd group (DVE).
"""
import numpy as np
import ml_dtypes
from contextlib import ExitStack

from concourse import bacc, bass, mybir, tile
from concourse.bass_utils import run_bass_kernel_spmd

N_CORES = 8
T, P, I, O, N_CH = 1024, 256, 256, 256, 1024
T_SH = T // N_CORES
OMEGA = 30.0
TWO_PI = float(2 * np.pi)
C_MAGIC = float(1.5 * 2**23)

BF16 = mybir.dt.bfloat16
F32 = mybir.dt.float32
I32 = mybir.dt.int32
FCOLS = T_SH * 512

U_PAD = 88           # compact weight stream capacity (slots of 512 cols)
CHUNK = 4            # slots per predicated chunk load
N_CHUNK = U_PAD // CHUNK


def build_nc(repeat=1, g=4, x_bufs=6, o_bufs=3,
             u_bufs=4, d_bufs=2, psum_bufs=2,
             out_engine="scalar", ld_engine="sync",
             out_bf16=True, u_dve_every=3, ld_span=1, dma_only=False,
             unroll=1, no_bias=False, static_slot=False,
             chunk_engine="sync", v_engine="vector",
             u_engine=("scalar", "scalar", "vector"),
             d_bf16=True, mm_only=False, pw_only=False):
    G = g
    N_G = T_SH // G
    nc = bacc.Bacc(None, target_bir_lowering=False)
    xT = nc.declare_dram_parameter("xT", [128, FCOLS], BF16, isOutput=False)
    wu = nc.declare_dram_parameter("wu", [128, U_PAD * 512], BF16,
                                   isOutput=False)
    bt = nc.declare_dram_parameter("bt", [128, g * 512], F32, isOutput=False)
    st = nc.declare_dram_parameter("st", [1, T_SH], I32, isOutput=False)
    cf = nc.declare_dram_parameter("cf", [1, N_CHUNK], I32, isOutput=False)
    out_dt = BF16 if out_bf16 else F32
    if dma_only:
        out = nc.declare_dram_parameter("out", [128, FCOLS], BF16,
                                        isOutput=True)
    else:
        out = nc.declare_dram_parameter(
            "out", [128, T_SH, 2, 256], out_dt, isOutput=True)

    with tile.TileContext(nc) as tc, ExitStack() as ctx:
        const_pool = ctx.enter_context(tc.tile_pool(name="const", bufs=1))
        x_pool = ctx.enter_context(tc.tile_pool(name="x", bufs=x_bufs))
        u_pool = ctx.enter_context(tc.tile_pool(name="u", bufs=u_bufs))
        d_pool = ctx.enter_context(tc.tile_pool(name="d", bufs=d_bufs))
        o_pool = ctx.enter_context(tc.tile_pool(name="o", bufs=o_bufs))
        psum_pool = ctx.enter_context(
            tc.tile_pool(name="psum", bufs=psum_bufs, space="PSUM"))

        bt_sb = const_pool.tile([128, g * 512], F32)
        nc.sync.dma_start(bt_sb[:], bt[:])
        st_sb = const_pool.tile([1, T_SH], I32)
        nc.sync.dma_start(st_sb[:], st[:])
        cf_sb = const_pool.tile([1, N_CHUNK], I32)
        nc.sync.dma_start(cf_sb[:], cf[:])
        c_magic = const_pool.tile([128, 1], F32)
        nc.gpsimd.memset(c_magic[:], C_MAGIC)
        wu_sb = const_pool.tile([128, U_PAD * 512], BF16)
        dummy = dummy_o = None
        if mm_only or pw_only:
            dummy = const_pool.tile([128, g, 2, 256], F32)
            nc.gpsimd.memset(dummy[:], 0.25)
            dummy_o = const_pool.tile([128, g, 2, 256], out_dt)
            nc.gpsimd.memset(dummy_o[:], 0.25)

        reg_n = [0]
        CH_ENG = {"sync": mybir.EngineType.SP,
                  "scalar": mybir.EngineType.Activation,
                  "gpsimd": mybir.EngineType.Pool}

        def chunk_load(c, eng="scalar"):
            reg_n[0] += 1
            reg = nc.alloc_registers(f"cf{reg_n[0]}", [CH_ENG[eng]])
            nc.regs_load(reg, cf_sb[0:1, c : c + 1])
            cond = nc.snap(reg, donate=True, min_val=0, max_val=1)
            cols = slice(c * CHUNK * 512, (c + 1) * CHUNK * 512)
            getattr(nc, eng).dma_start(wu_sb[:, cols], wu[:, cols], cond=cond)

        pending = []  # [(ob, dB, t0)] sin+store lagged one group

        def flush_pending():
            item = pending.pop()
            if len(item) == 4:  # mm_only: ob pre-filled
                _, _, t0, ob = item
            else:
                ob_t, dB, t0 = item
                ob = ob_t
                nc.scalar.activation(
                    ob[:], dB[:], mybir.ActivationFunctionType.Sin,
                    scale=-TWO_PI)
            getattr(nc, out_engine).dma_start(
                out[:, t0 : t0 + G], ob[:])

        ld_state = {}

        def group_body(gi):
            t0 = gi * G
            cols = slice(t0 * 512, (t0 + G) * 512)
            if gi + 1 < N_CHUNK:
                chunk_load(gi + 1, eng=chunk_engine)
            if gi % ld_span == 0:
                spc = slice(t0 * 512, (t0 + ld_span * G) * 512)
                xbig = x_pool.tile([128, ld_span * G * 512], BF16)
                getattr(nc, ld_engine).dma_start(xbig[:], xT[:, spc])
                ld_state["x"] = xbig
            off = (gi % ld_span) * G * 512
            xb = ld_state["x"]
            if dma_only:
                getattr(nc, out_engine).dma_start(
                    out[:, cols], xb[:, off : off + G * 512])
                return

            psum = psum_pool.tile([128, G, 2, 256], F32)
            for j in range(G):
                if pw_only:
                    break
                p = t0 + j
                if static_slot:
                    sv = None
                else:
                    # st holds slot*512 element offsets (host-precomputed)
                    sv = nc.values_load(
                        st_sb[0:1, p : p + 1],
                        engines=[mybir.EngineType.PE],
                        min_val=0, max_val=(U_PAD - 1) * 512,
                        skip_runtime_bounds_check=True)
                trk = min(p, U_PAD - 1) * 512
                for h in range(2):
                    c0 = off + j * 512
                    for k in range(2):
                        if static_slot:
                            cw = (p % U_PAD) * 512 + k * 256
                            rhs = wu_sb[:, cw : cw + 256]
                        else:
                            a = wu_sb[:, bass.ds(sv + k * 256, 256)]
                            # slots are monotone (st[p] <= p), so tracking
                            # the read at slot min(p, U_PAD-1) is safe and
                            # narrows the dep to one chunk load.
                            rhs = bass.AP(
                                tensor=a.tensor, offset=a.offset, ap=a.ap,
                                dep_tracking_offset=trk + k * 256)
                        nc.tensor.matmul(
                            psum[:, j, h, :],
                            xb[:, c0 + 256 * k + 128 * h :
                                  c0 + 256 * k + 128 * (h + 1)],
                            rhs,
                            start=(k == 0),
                            stop=(k == 1),
                        )
            if mm_only:
                pending.append((None, None, t0, dummy_o))
                return
            src = dummy if pw_only else psum
            if no_bias:
                vB = src
            else:
                vB = u_pool.tile([128, G, 2, 256], F32)
                getattr(nc, v_engine).tensor_tensor(
                    vB[:], src[:], bt_sb[:], mybir.AluOpType.add)
            uB = u_pool.tile([128, G, 2, 256], F32)
            ue = (u_engine if not isinstance(u_engine, (list, tuple))
                  else u_engine[gi % len(u_engine)])
            if ue == "scalar":
                nc.scalar.activation(
                    uB[:], vB[:], mybir.ActivationFunctionType.Identity,
                    bias=c_magic[:], scale=1.0)
            else:
                getattr(nc, ue).tensor_scalar(
                    uB[:], vB[:], C_MAGIC, None, mybir.AluOpType.add)
            dB = d_pool.tile([128, G, 2, 256], BF16 if d_bf16 else F32)
            nc.vector.scalar_tensor_tensor(
                dB[:], uB[:], C_MAGIC, vB[:],
                mybir.AluOpType.subtract, mybir.AluOpType.subtract)
            ob = o_pool.tile([128, G, 2, 256], out_dt)
            pending.append((ob, dB, t0))

        def full_body(_iv=None):
            chunk_load(0, eng=chunk_engine)
            for gi in range(N_G):
                group_body(gi)
                if len(pending) > 1:
                    flush_pending()
            while pending:
                flush_pending()

        if repeat == 1:
            for _ in range(unroll):
                full_body()
        else:
            assert repeat % unroll == 0
            with tc.For_i(0, repeat // unroll, 1):
                for _ in range(unroll):
                    full_body()

    nc.compile()
    return nc


_NC = None


def _get_nc():
    global _NC
    if _NC is None:
        _NC = build_nc()
    return _NC


def _schedule(indices):
    """Sort tiles by channel globally, deal contiguous 128-tile blocks to
    cores. Per core: sched (original tile ids in schedule order), slot id
    per position (first-use order), unique channel list."""
    order = np.argsort(indices, kind="stable")
    scheds, slots, uniqs = [], [], []
    for c in range(N_CORES):
        blk = order[c * T_SH : (c + 1) * T_SH]
        ch = indices[blk]
        slot = np.empty(T_SH, np.int32)
        uniq = []
        for p in range(T_SH):
            if p == 0 or ch[p] != ch[p - 1]:
                uniq.append(ch[p])
            slot[p] = (len(uniq) - 1) * 512  # element offset of the slot
        scheds.append(blk)
        slots.append(slot)
        uniqs.append(np.asarray(uniq, np.int64))
    return scheds, slots, uniqs


_SCHEDS = None  # stashed by make_in_maps for unshard


def make_in_maps(x, weight, bias, indices):
    global _SCHEDS
    x = np.asarray(x, dtype=np.float32)
    weight = np.asarray(weight, dtype=np.float32)
    bias = np.asarray(bias, dtype=np.float32).reshape(O)
    indices = np.asarray(indices).astype(np.int64)

    bprime = (OMEGA / TWO_PI * bias).astype(np.float32)  # [O]
    bt_h = np.ascontiguousarray(
        np.broadcast_to(np.tile(bprime, 4 * 2), (128, 4 * 512))
    ).astype(np.float32)
    wsc = (OMEGA / TWO_PI * weight).astype(np.float32)

    scheds, slots, uniqs = _schedule(indices)
    assert max(len(u) for u in uniqs) <= U_PAD, (
        "unique-channel overflow; pathological index distribution")
    _SCHEDS = scheds
    in_maps = []
    for c in range(N_CORES):
        sched = scheds[c]
        xT_h = (
            np.ascontiguousarray(
                x[sched].reshape(T_SH, P, 2, 128).transpose(3, 0, 2, 1))
            .astype(ml_dtypes.bfloat16)
            .reshape(128, FCOLS)
        )
        uq = uniqs[c]
        wu_h = np.zeros((128, U_PAD * 512), ml_dtypes.bfloat16)
        ws = wsc[uq]  # [U_c, I, O]
        wu_h[:, : len(uq) * 512] = (
            np.ascontiguousarray(
                ws.reshape(len(uq), 2, 128, O).transpose(2, 0, 1, 3))
            .astype(ml_dtypes.bfloat16)
            .reshape(128, len(uq) * 512)
        )
        n_chunk = -(-len(uq) // CHUNK)
        cf_h = np.zeros((1, N_CHUNK), np.int32)
        cf_h[0, :n_chunk] = 1
        in_maps.append({"xT": xT_h, "wu": wu_h, "bt": bt_h,
                        "st": slots[c].reshape(1, T_SH), "cf": cf_h})
    return in_maps


def unshard(results):
    out = np.empty((T, P, O), np.float32)
    for c, r in enumerate(results):
        o = np.asarray(r["out"]).astype(np.float32)  # [128, T_SH, 2, 256]
        # dims: (point-in-half, tile, point-half, feature)
        o = o.transpose(1, 2, 0, 3).reshape(T_SH, P, O)
        out[_SCHEDS[c]] = o
    return out


def kernel(x, weight, bias, indices):
    nc = _get_nc()
    in_maps = make_in_maps(x, weight, bias, indices)
    try:
        res = run_bass_kernel_spmd(nc, in_maps, core_ids=list(range(N_CORES)))
    except ModuleNotFoundError:
        import os

        os.environ["BASS_NEVER_TRACE"] = "1"
        res = run_bass_kernel_spmd(nc, in_maps, core_ids=list(range(N_CORES)))
    return unshard(res.results)


# revision 54
# speedup vs baseline: 6.1224x; 1.1129x over previous
"""AdaptiveSinLayer kernel for 8 TRN2 NeuronCores (data-parallel).

out[t] = sin(OMEGA*(x[t] @ weight[indices[t]] + bias)).

Same math as v7 (weights pre-scaled by OMEGA/2pi so the sine period in
psum units is exactly 1):
  u = round(z') + C   (magic add)
  d = (u - C) - z'    (fused scalar_tensor_tensor)
  o = Sin(-2pi*d + b30vec)
but the pointwise ops run over a whole group of G=4 tiles at once: one
[128, G*512] PSUM access pattern (4 banks) per group amortizes the
~200ns/instr engine overhead 4x. The Sin (+ output DMA) for group g is
issued after u/d of group g+1, so ACT's strict FIFO never waits on the
DVE chain. u runs on ACT except every u_dve_every-th group (DVE),
balancing ACT ~= DVE ~= 91us, both under the ~140us DMA floor.
"""
import numpy as np
import ml_dtypes
from contextlib import ExitStack

from concourse import bacc, mybir, tile
from concourse.bass_utils import run_bass_kernel_spmd

N_CORES = 8
T, P, I, O, N_CH = 1024, 256, 256, 256, 1024
T_SH = T // N_CORES
OMEGA = 30.0
TWO_PI = float(2 * np.pi)
C_MAGIC = float(1.5 * 2**23)

BF16 = mybir.dt.bfloat16
F32 = mybir.dt.float32
FCOLS = T_SH * 512


def build_nc(repeat=1, g=4, x_bufs=6, w_bufs=6, o_bufs=4,
             u_bufs=3, d_bufs=3, psum_bufs=2,
             out_engine="scalar", ld_engine="sync",
             out_bf16=True, u_dve_every=3, ld_span=1, dma_only=False,
             lag=1, unroll=1):
    G = g
    N_G = T_SH // G
    nc = bacc.Bacc(None, target_bir_lowering=False)
    xT = nc.declare_dram_parameter("xT", [128, FCOLS], BF16, isOutput=False)
    wg = nc.declare_dram_parameter("wg", [128, FCOLS], BF16, isOutput=False)
    bv = nc.declare_dram_parameter("bv", [128, 2], F32, isOutput=False)
    out_dt = BF16 if out_bf16 else F32
    if dma_only:
        out = nc.declare_dram_parameter("out", [128, FCOLS], BF16,
                                        isOutput=True)
    else:
        out = nc.declare_dram_parameter(
            "out", [128, T_SH, 2, 256], out_dt, isOutput=True)

    with tile.TileContext(nc) as tc, ExitStack() as ctx:
        const_pool = ctx.enter_context(tc.tile_pool(name="const", bufs=1))
        x_pool = ctx.enter_context(tc.tile_pool(name="x", bufs=x_bufs))
        w_pool = ctx.enter_context(tc.tile_pool(name="w", bufs=w_bufs))
        u_pool = ctx.enter_context(tc.tile_pool(name="u", bufs=u_bufs))
        d_pool = ctx.enter_context(tc.tile_pool(name="d", bufs=d_bufs))
        o_pool = ctx.enter_context(tc.tile_pool(name="o", bufs=o_bufs))
        psum_pool = ctx.enter_context(
            tc.tile_pool(name="psum", bufs=psum_bufs, space="PSUM"))

        bv_sb = const_pool.tile([128, 2], F32)
        nc.sync.dma_start(bv_sb[:], bv[:])
        c_magic = const_pool.tile([128, 1], F32)
        nc.gpsimd.memset(c_magic[:], C_MAGIC)

        pending = []  # [(ob, dB, t0)] sin+store lagged one group

        def flush_pending():
            ob, dB, t0 = pending.pop()
            for m in range(2):
                nc.scalar.activation(
                    ob[:, :, m, :], dB[:, :, m, :],
                    mybir.ActivationFunctionType.Sin,
                    bias=bv_sb[:, m : m + 1], scale=-TWO_PI)
            getattr(nc, out_engine).dma_start(
                out[:, t0 : t0 + G], ob[:])

        ld_state = {}

        def group_body(gi):
            t0 = gi * G
            cols = slice(t0 * 512, (t0 + G) * 512)
            if gi % ld_span == 0:
                spc = slice(t0 * 512, (t0 + ld_span * G) * 512)
                xbig = x_pool.tile([128, ld_span * G * 512], BF16)
                getattr(nc, ld_engine).dma_start(xbig[:], xT[:, spc])
                wbig = w_pool.tile([128, ld_span * G * 512], BF16)
                getattr(nc, ld_engine).dma_start(wbig[:], wg[:, spc])
                ld_state["x"], ld_state["w"] = xbig, wbig
            off = (gi % ld_span) * G * 512
            xb, wb = ld_state["x"], ld_state["w"]
            if dma_only:
                getattr(nc, out_engine).dma_start(
                    out[:, cols], xb[:, off : off + G * 512])
                return

            psum = psum_pool.tile([128, G, 2, 256], F32)
            for j in range(G):
                for m in range(2):
                    for k in range(2):
                        c0 = off + j * 512 + 256 * k
                        nc.tensor.matmul(
                            psum[:, j, m, :],
                            wb[:, c0 + 128 * m : c0 + 128 * (m + 1)],
                            xb[:, c0 : c0 + 256],
                            start=(k == 0),
                            stop=(k == 1),
                        )
            uB = u_pool.tile([128, G, 2, 256], F32)
            if u_dve_every and (gi % u_dve_every == u_dve_every - 1):
                nc.vector.tensor_scalar(
                    uB[:], psum[:], C_MAGIC, None, mybir.AluOpType.add)
            else:
                nc.scalar.activation(
                    uB[:], psum[:], mybir.ActivationFunctionType.Identity,
                    bias=c_magic[:], scale=1.0)
            dB = d_pool.tile([128, G, 2, 256], F32)
            nc.vector.scalar_tensor_tensor(
                dB[:], uB[:], C_MAGIC, psum[:],
                mybir.AluOpType.subtract, mybir.AluOpType.subtract)
            ob = o_pool.tile([128, G, 2, 256], out_dt)
            pending.append((ob, dB, t0))

        def full_body(_iv=None):
            for gi in range(N_G):
                group_body(gi)
                if len(pending) > 1:
                    flush_pending()
            while pending:
                flush_pending()

        if repeat == 1:
            for _ in range(unroll):
                full_body()
        else:
            assert repeat % unroll == 0
            with tc.For_i(0, repeat // unroll, 1):
                for _ in range(unroll):
                    full_body()

    nc.compile()
    return nc


_NC = None


def _get_nc():
    global _NC
    if _NC is None:
        _NC = build_nc()
    return _NC


def make_in_maps(x, weight, bias, indices):
    x = np.asarray(x, dtype=np.float32)
    weight = np.asarray(weight, dtype=np.float32)
    bias = np.asarray(bias, dtype=np.float32).reshape(O)
    indices = np.asarray(indices).astype(np.int64)

    bv_h = np.ascontiguousarray(
        (OMEGA * bias).reshape(2, 128).T).astype(np.float32)

    wsc = (OMEGA / TWO_PI * weight).astype(np.float32)
    in_maps = []
    for c in range(N_CORES):
        sl = slice(c * T_SH, (c + 1) * T_SH)
        xT_h = (
            np.ascontiguousarray(
                x[sl].reshape(T_SH, P, 2, 128).transpose(3, 0, 2, 1))
            .astype(ml_dtypes.bfloat16)
            .reshape(128, FCOLS)
        )
        ws = wsc[indices[sl]]
        wg_h = (
            np.ascontiguousarray(
                ws.reshape(T_SH, 2, 128, O).transpose(2, 0, 1, 3))
            .astype(ml_dtypes.bfloat16)
            .reshape(128, FCOLS)
        )
        in_maps.append({"xT": xT_h, "wg": wg_h, "bv": bv_h})
    return in_maps


def unshard(results):
    outs = []
    for r in results:
        o = np.asarray(r["out"]).astype(np.float32)  # [128, T_SH, 2, 256]
        o = o.transpose(1, 3, 2, 0).reshape(T_SH, P, O)
        outs.append(o)
    return np.concatenate(outs, axis=0)


def kernel(x, weight, bias, indices):
    nc = _get_nc()
    in_maps = make_in_maps(x, weight, bias, indices)
    try:
        res = run_bass_kernel_spmd(nc, in_maps, core_ids=list(range(N_CORES)))
    except ModuleNotFoundError:
        import os

        os.environ["BASS_NEVER_TRACE"] = "1"
        res = run_bass_kernel_spmd(nc, in_maps, core_ids=list(range(N_CORES)))
    return unshard(res.results)



# revision 55
# speedup vs baseline: 6.4812x; 1.0586x over previous
"""AdaptiveSinLayer kernel for 8 TRN2 NeuronCores (data-parallel).

out[t] = sin(OMEGA*(x[t] @ weight[indices[t]] + bias)).

Same math as v7 (weights pre-scaled by OMEGA/2pi so the sine period in
psum units is exactly 1):
  u = round(z') + C   (magic add)
  d = (u - C) - z'    (fused scalar_tensor_tensor)
  o = Sin(-2pi*d + b30vec)
but the pointwise ops run over a whole group of G=4 tiles at once: one
[128, G*512] PSUM access pattern (4 banks) per group amortizes the
~200ns/instr engine overhead 4x. The Sin (+ output DMA) for group g is
issued after u/d of group g+1, so ACT's strict FIFO never waits on the
DVE chain. u runs on ACT except every u_dve_every-th group (DVE),
balancing ACT ~= DVE ~= 91us, both under the ~140us DMA floor.
"""
import numpy as np
import ml_dtypes
from contextlib import ExitStack

from concourse import bacc, mybir, tile
from concourse.bass_utils import run_bass_kernel_spmd

N_CORES = 8
T, P, I, O, N_CH = 1024, 256, 256, 256, 1024
T_SH = T // N_CORES
OMEGA = 30.0
TWO_PI = float(2 * np.pi)
C_MAGIC = float(1.5 * 2**23)

BF16 = mybir.dt.bfloat16
F32 = mybir.dt.float32
FCOLS = T_SH * 512


def build_nc(repeat=1, g=4, x_bufs=6, w_bufs=6, o_bufs=4,
             u_bufs=3, d_bufs=3, psum_bufs=2,
             out_engine="gpsimd", ld_engine="sync",
             out_bf16=True, u_dve_every=3, ld_span=1, dma_only=False,
             lag=1, unroll=1):
    G = g
    N_G = T_SH // G
    nc = bacc.Bacc(None, target_bir_lowering=False)
    xT = nc.declare_dram_parameter("xT", [128, FCOLS], BF16, isOutput=False)
    wg = nc.declare_dram_parameter("wg", [128, FCOLS], BF16, isOutput=False)
    bv = nc.declare_dram_parameter("bv", [128, 2], F32, isOutput=False)
    out_dt = BF16 if out_bf16 else F32
    if dma_only:
        out = nc.declare_dram_parameter("out", [128, FCOLS], BF16,
                                        isOutput=True)
    else:
        out = nc.declare_dram_parameter(
            "out", [128, T_SH, 2, 256], out_dt, isOutput=True)

    with tile.TileContext(nc) as tc, ExitStack() as ctx:
        const_pool = ctx.enter_context(tc.tile_pool(name="const", bufs=1))
        x_pool = ctx.enter_context(tc.tile_pool(name="x", bufs=x_bufs))
        w_pool = ctx.enter_context(tc.tile_pool(name="w", bufs=w_bufs))
        u_pool = ctx.enter_context(tc.tile_pool(name="u", bufs=u_bufs))
        d_pool = ctx.enter_context(tc.tile_pool(name="d", bufs=d_bufs))
        o_pool = ctx.enter_context(tc.tile_pool(name="o", bufs=o_bufs))
        psum_pool = ctx.enter_context(
            tc.tile_pool(name="psum", bufs=psum_bufs, space="PSUM"))

        bv_sb = const_pool.tile([128, 2], F32)
        nc.sync.dma_start(bv_sb[:], bv[:])
        c_magic = const_pool.tile([128, 1], F32)
        nc.gpsimd.memset(c_magic[:], C_MAGIC)

        pending = []  # [(ob, dB, t0)] sin+store lagged one group

        def flush_pending():
            ob, dB, t0 = pending.pop()
            for m in range(2):
                nc.scalar.activation(
                    ob[:, :, m, :], dB[:, :, m, :],
                    mybir.ActivationFunctionType.Sin,
                    bias=bv_sb[:, m : m + 1], scale=-TWO_PI)
            getattr(nc, out_engine).dma_start(
                out[:, t0 : t0 + G], ob[:])

        ld_state = {}

        def group_body(gi):
            t0 = gi * G
            cols = slice(t0 * 512, (t0 + G) * 512)
            if gi % ld_span == 0:
                spc = slice(t0 * 512, (t0 + ld_span * G) * 512)
                xbig = x_pool.tile([128, ld_span * G * 512], BF16)
                getattr(nc, ld_engine).dma_start(xbig[:], xT[:, spc])
                wbig = w_pool.tile([128, ld_span * G * 512], BF16)
                getattr(nc, ld_engine).dma_start(wbig[:], wg[:, spc])
                ld_state["x"], ld_state["w"] = xbig, wbig
            off = (gi % ld_span) * G * 512
            xb, wb = ld_state["x"], ld_state["w"]
            if dma_only:
                getattr(nc, out_engine).dma_start(
                    out[:, cols], xb[:, off : off + G * 512])
                return

            psum = psum_pool.tile([128, G, 2, 256], F32)
            for j in range(G):
                for m in range(2):
                    for k in range(2):
                        c0 = off + j * 512 + 256 * k
                        nc.tensor.matmul(
                            psum[:, j, m, :],
                            wb[:, c0 + 128 * m : c0 + 128 * (m + 1)],
                            xb[:, c0 : c0 + 256],
                            start=(k == 0),
                            stop=(k == 1),
                        )
            uB = u_pool.tile([128, G, 2, 256], F32)
            if u_dve_every and (gi % u_dve_every == u_dve_every - 1):
                nc.vector.tensor_scalar(
                    uB[:], psum[:], C_MAGIC, None, mybir.AluOpType.add)
            else:
                nc.scalar.activation(
                    uB[:], psum[:], mybir.ActivationFunctionType.Identity,
                    bias=c_magic[:], scale=1.0)
            dB = d_pool.tile([128, G, 2, 256], F32)
            nc.vector.scalar_tensor_tensor(
                dB[:], uB[:], C_MAGIC, psum[:],
                mybir.AluOpType.subtract, mybir.AluOpType.subtract)
            ob = o_pool.tile([128, G, 2, 256], out_dt)
            pending.append((ob, dB, t0))

        def full_body(_iv=None):
            for gi in range(N_G):
                group_body(gi)
                if len(pending) > 1:
                    flush_pending()
            while pending:
                flush_pending()

        if repeat == 1:
            for _ in range(unroll):
                full_body()
        else:
            assert repeat % unroll == 0
            with tc.For_i(0, repeat // unroll, 1):
                for _ in range(unroll):
                    full_body()

    nc.compile()
    return nc


_NC = None


def _get_nc():
    global _NC
    if _NC is None:
        _NC = build_nc()
    return _NC


def make_in_maps(x, weight, bias, indices):
    x = np.asarray(x, dtype=np.float32)
    weight = np.asarray(weight, dtype=np.float32)
    bias = np.asarray(bias, dtype=np.float32).reshape(O)
    indices = np.asarray(indices).astype(np.int64)

    bv_h = np.ascontiguousarray(
        (OMEGA * bias).reshape(2, 128).T).astype(np.float32)

    wsc = (OMEGA / TWO_PI * weight).astype(np.float32)
    in_maps = []
    for c in range(N_CORES):
        sl = slice(c * T_SH, (c + 1) * T_SH)
        xT_h = (
            np.ascontiguousarray(
                x[sl].reshape(T_SH, P, 2, 128).transpose(3, 0, 2, 1))
            .astype(ml_dtypes.bfloat16)
            .reshape(128, FCOLS)
        )
        ws = wsc[indices[sl]]
        wg_h = (
            np.ascontiguousarray(
                ws.reshape(T_SH, 2, 128, O).transpose(2, 0, 1, 3))
            .astype(ml_dtypes.bfloat16)
            .reshape(128, FCOLS)
        )
        in_maps.append({"xT": xT_h, "wg": wg_h, "bv": bv_h})
    return in_maps


def unshard(results):
    outs = []
    for r in results:
        o = np.asarray(r["out"]).astype(np.float32)  # [128, T_SH, 2, 256]
        o = o.transpose(1, 3, 2, 0).reshape(T_SH, P, O)
        outs.append(o)
    return np.concatenate(outs, axis=0)


def kernel(x, weight, bias, indices):
    nc = _get_nc()
    in_maps = make_in_maps(x, weight, bias, indices)
    try:
        res = run_bass_kernel_spmd(nc, in_maps, core_ids=list(range(N_CORES)))
    except ModuleNotFoundError:
        import os

        os.environ["BASS_NEVER_TRACE"] = "1"
        res = run_bass_kernel_spmd(nc, in_maps, core_ids=list(range(N_CORES)))
    return unshard(res.results)

